# revision 9
# baseline (speedup 1.0000x reference)
"""Trainium2 Bass kernel for BLSTMModel: 2-layer bidirectional LSTM + vocab projection.

Sharding: LSTM replicated on all 8 cores (recurrence is PE-streaming-bound and tiny-state);
FC vocab projection sharded across cores (fc_W rows / 8 = 4000 per core).

All matmuls in float32r (TF32-like, ~1.5e-4 rel err, 1 cyc/row at N>=256 vs fp32's 4 cyc).

Device-side layout conventions:
  - token order (t, b): token index = t*B + b
  - K-major operands stored [128, K/128, M] ("kxm" form)
  - h state kept transposed ([H, 8] per step) in a small SBUF tile, written through to
    DRAM buffers [128, KH, 8*T]; bwd-layer-1's DRAM buffer is written time-reversed so
    the FC can read both halves in plain t-order
  - bwd chains consume time-reversed embeddings (host-prepared), so they run in
    s = T-1-t processing order
"""

import os
import numpy as np

V, E, H, B, T = 32000, 512, 512, 8, 256
G = 4 * H            # 2048 gates
NTOK = B * T         # 2048 tokens
NCORES = 8
VSH = V // NCORES    # 4000 vocab rows per core
KE = E // 128        # 4
KH = H // 128        # 4

_CACHE = {}


def _build(nc_T):
    import concourse.mybir as mybir
    from concourse import bacc
    from concourse.tile import TileContext

    f32 = mybir.dt.float32
    f32r = mybir.dt.float32r
    assert nc_T % 16 == 0
    ntok = B * nc_T
    mt = ntok // 128

    nc = bacc.Bacc(None, target_bir_lowering=False)

    # ---------------- I/O ----------------
    embT = {d: nc.dram_tensor(f"embT_{d}", [128, KE, ntok], f32r, kind="ExternalInput")
            for d in ("f", "b")}
    CH = ["f0", "b0", "f1", "b1"]
    WihT = {c: nc.dram_tensor(f"wihT_{c}", [128, KE, G], f32r, kind="ExternalInput") for c in CH}
    WhhT = {c: nc.dram_tensor(f"whhT_{c}", [128, KH, G], f32r, kind="ExternalInput") for c in CH}
    bih = {c: nc.dram_tensor(f"bih_{c}", [128, G], f32, kind="ExternalInput") for c in CH}
    i8r = nc.dram_tensor("i8r", [8, 8], f32r, kind="ExternalInput")
    i8f = nc.dram_tensor("i8f", [8, 8], f32, kind="ExternalInput")
    zcol = nc.dram_tensor("zcol", [128, KH, 8], f32r, kind="ExternalInput")
    fcWT = nc.dram_tensor("fcWT", [128, 2 * KH, VSH], f32r, kind="ExternalInput")
    fcb = nc.dram_tensor("fcb", [128, VSH], f32, kind="ExternalInput")

    logits = nc.dram_tensor("logits", [B * nc_T, VSH], f32, kind="ExternalOutput")
    hT_o = nc.dram_tensor("hT_o", [2, B, H], f32, kind="ExternalOutput")
    cT_o = nc.dram_tensor("cT_o", [2, B, H], f32, kind="ExternalOutput")

    Sig = mybir.ActivationFunctionType.Sigmoid
    Tanh = mybir.ActivationFunctionType.Tanh
    Copy = mybir.ActivationFunctionType.Copy

    with TileContext(nc) as tc:
        from contextlib import ExitStack
        es = ExitStack()
        with es:
            const = es.enter_context(tc.tile_pool(name="const", bufs=1))
            dram = es.enter_context(tc.tile_pool(name="dram", bufs=1, space="DRAM"))
            t_i8r = const.tile([8, 8], f32r, tag="i8r", name="t_i8r")
            t_i8f = const.tile([8, 8], f32, tag="i8f", name="t_i8f")
            nc.sync.dma_start(t_i8r[:], i8r[:])
            nc.sync.dma_start(t_i8f[:], i8f[:])

            # DRAM scratch: xpart (+bias) rows in processing order; hT buffers kxm-form
            xp = {c: dram.tile([ntok, G], f32r, tag=f"xp_{c}", name=f"xp_{c}") for c in CH}
            htd = {c: dram.tile([128, KH, 8 * nc_T], f32r, tag=f"htd_{c}", name=f"htd_{c}")
                   for c in CH}

            # ---------- helper: xpart GEMM: out[m,:] = kxm.T @ WT + bias ----------
            def xpart_gemm(kxm_get, wT_dram, bias_dram, out_dram, wpool, gpool, gps):
                wsb = wpool.tile([128, KE, G], f32r, tag="w_gemm", name="w_gemm")
                bsb = wpool.tile([128, G], f32, tag="b_gemm", name="b_gemm")
                nc.sync.dma_start(wsb[:], wT_dram[:])
                nc.sync.dma_start(bsb[:], bias_dram[:])
                for m in range(mt):
                    kxm = kxm_get(m)
                    osb = gpool.tile([128, G], f32r, tag="o_gemm", name="o_gemm")
                    for n in range(G // 512):
                        ps = gps.tile([128, 512], f32, tag="ps_gemm", name="ps_gemm")
                        for k in range(KE):
                            nc.tensor.matmul(
                                ps[:], kxm[:, k, :], wsb[:, k, n * 512:(n + 1) * 512],
                                start=(k == 0), stop=(k == KE - 1))
                        nc.vector.tensor_add(out=osb[:, n * 512:(n + 1) * 512], in0=ps[:],
                                             in1=bsb[:, n * 512:(n + 1) * 512])
                    nc.sync.dma_start(out_dram[m * 128:(m + 1) * 128, :], osb[:])

            def sbuf_kxm(buf):
                return lambda m: buf[:, :, m * 128:(m + 1) * 128]

            def dram_kxm(dbuf, pool, nm):
                def get(m):
                    t = pool.tile([128, KH, 128], f32r, tag=f"kxm_{nm}", name=f"kxm_{nm}")
                    nc.sync.dma_start(t[:], dbuf[:, :, m * 128:(m + 1) * 128])
                    return t
                return get

            # ---------- helper: one recurrence phase (two chains in lockstep) ----------
            def recurrence(chains, pools):
                """chains: (name, whh_sb, reverse_out, layer_or_None)."""
                work, xpool, rps, spool = pools
                c_prev, h_prev = {}, {}
                for name, _, _, _ in chains:
                    h0 = spool.tile([128, KH, 8], f32r, tag=f"hs_{name}", name=f"h0_{name}")
                    nc.sync.dma_start(h0[:], zcol[:])
                    h_prev[name] = h0
                    c0 = work.tile([8, H], f32, tag=f"c_{name}", name=f"c0_{name}")
                    nc.vector.memset(c0[:], 0.0)
                    c_prev[name] = c0
                for t in range(nc_T):
                    for name, whh_sb, rev, layer in chains:
                        xt = xpool.tile([8, G], f32r, tag=f"xt_{name}", name=f"xt_{name}")
                        nc.sync.dma_start(xt[:], xp[name][t * 8:(t + 1) * 8, :])
                        gp = rps.tile([8, G], f32, tag=f"gp_{name}", name=f"gp_{name}")
                        for n in range(G // 512):
                            for k in range(KH):
                                nc.tensor.matmul(
                                    gp[:, n * 512:(n + 1) * 512],
                                    h_prev[name][:, k, :], whh_sb[:, k, n * 512:(n + 1) * 512],
                                    start=(k == 0), stop=False)
                            nc.tensor.matmul(
                                gp[:, n * 512:(n + 1) * 512],
                                t_i8r[:], xt[:, n * 512:(n + 1) * 512],
                                start=False, stop=True)
                        # elementwise: gate order i, f, g, o
                        if_sb = work.tile([8, 2 * H], f32, tag=f"if_{name}", name=f"if_{name}")
                        g_sb = work.tile([8, H], f32, tag=f"g_{name}", name=f"g_{name}")
                        o_sb = work.tile([8, H], f32, tag=f"o_{name}", name=f"o_{name}")
                        nc.scalar.activation(if_sb[:], gp[:, 0:2 * H], Sig)
                        nc.scalar.activation(g_sb[:], gp[:, 2 * H:3 * H], Tanh)
                        nc.scalar.activation(o_sb[:], gp[:, 3 * H:4 * H], Sig)
                        t1 = work.tile([8, H], f32, tag=f"t1_{name}", name=f"t1_{name}")
                        c_new = work.tile([8, H], f32, tag=f"c_{name}", name=f"cn_{name}")
                        nc.vector.tensor_mul(out=t1[:], in0=if_sb[:, H:2 * H], in1=c_prev[name][:])
                        nc.vector.tensor_mul(out=g_sb[:], in0=if_sb[:, 0:H], in1=g_sb[:])
                        nc.vector.tensor_add(out=c_new[:], in0=t1[:], in1=g_sb[:])
                        c_prev[name] = c_new
                        nc.scalar.activation(t1[:], c_new[:], Tanh)
                        h_sb = work.tile([8, H], f32, tag=f"h_{name}", name=f"h_{name}")
                        nc.vector.tensor_mul(out=h_sb[:], in0=o_sb[:], in1=t1[:])
                        # transpose h -> [H, 8] psum; copy to new state tile; write-through DRAM
                        hp = rps.tile([128, KH * 8], f32, tag=f"gp_{name}", name=f"hp_{name}")
                        for k in range(KH):
                            nc.tensor.transpose(hp[:, k * 8:(k + 1) * 8],
                                                h_sb[:, k * 128:(k + 1) * 128], t_i8f[:])
                        h_new = spool.tile([128, KH, 8], f32r, tag=f"hs_{name}",
                                           name=f"hn_{name}")
                        nc.scalar.activation(h_new[:], hp[:].rearrange("p (k b) -> p k b", k=KH),
                                             Copy)
                        h_prev[name] = h_new
                        col = (nc_T - 1 - t) if rev else t
                        nc.sync.dma_start(htd[name][:, :, col * 8:(col + 1) * 8], h_new[:])
                        if layer is not None and t == nc_T - 1:
                            nc.sync.dma_start(hT_o[layer, :, :], h_sb[:])
                            nc.sync.dma_start(cT_o[layer, :, :], c_new[:])

            def gemm_phase(cf, cb, kxm_f, kxm_b):
                with (
                    tc.tile_pool(name="gw", bufs=1) as wpool,
                    tc.tile_pool(name="go", bufs=3) as gpool,
                    tc.tile_pool(name="gps", bufs=8, space="PSUM") as gps,
                ):
                    xpart_gemm(kxm_f, WihT[cf], bih[cf], xp[cf], wpool, gpool, gps)
                    xpart_gemm(kxm_b, WihT[cb], bih[cb], xp[cb], wpool, gpool, gps)

            def rec_phase(cf, cb):
                with (
                    tc.tile_pool(name="whh", bufs=1) as whhp,
                    tc.tile_pool(name="rwork", bufs=2) as work,
                    tc.tile_pool(name="rx", bufs=3) as xpool,
                    tc.tile_pool(name="rst", bufs=2) as spool,
                    tc.tile_pool(name="rps", bufs=1, space="PSUM") as rps,
                ):
                    wh = {}
                    for cn in (cf, cb):
                        wh[cn] = whhp.tile([128, KH, G], f32r, tag=f"whh_{cn}", name=f"whh_{cn}")
                        nc.sync.dma_start(wh[cn][:], WhhT[cn][:])
                    layer = 0 if cf == "f0" else 1
                    recurrence([(cf, wh[cf], False, layer),
                                (cb, wh[cb], cb == "b1", None)],
                               (work, xpool, rps, spool))

            # ================= Phase A: layer 0 =================
            with tc.tile_pool(name="emb", bufs=1) as embp:
                esb = {}
                for d in ("f", "b"):
                    esb[d] = embp.tile([128, KE, ntok], f32r, tag=f"emb_{d}", name=f"emb_{d}")
                    nc.sync.dma_start(esb[d][:], embT[d][:])
                gemm_phase("f0", "b0", sbuf_kxm(esb["f"]), sbuf_kxm(esb["b"]))
            rec_phase("f0", "b0")

            # ================= Phase B: layer 1 =================
            with tc.tile_pool(name="kxmp", bufs=3) as kxmp:
                gemm_phase("f1", "b1",
                           dram_kxm(htd["f0"], kxmp, "f"), dram_kxm(htd["b0"], kxmp, "b"))
            rec_phase("f1", "b1")

            # ================= Phase C: FC =================
            with (
                tc.tile_pool(name="fcht", bufs=1) as fchtp,
                tc.tile_pool(name="fcw", bufs=2) as fcwp,
                tc.tile_pool(name="fcc", bufs=1) as fccp,
                tc.tile_pool(name="fco", bufs=3) as fcop,
                tc.tile_pool(name="fps", bufs=8, space="PSUM") as fps,
            ):
                htf = fchtp.tile([128, KH, 8 * nc_T], f32r, tag="htf", name="htf")
                htb = fchtp.tile([128, KH, 8 * nc_T], f32r, tag="htb", name="htb")
                nc.sync.dma_start(htf[:], htd["f1"][:])
                nc.sync.dma_start(htb[:], htd["b1"][:])
                fcb_sb = fccp.tile([128, VSH], f32, tag="fcb", name="fcb_sb")
                nc.sync.dma_start(fcb_sb[:], fcb[:])
                n_chunks = [(i * 512, min(512, VSH - i * 512)) for i in range((VSH + 511) // 512)]
                for (n0, nsz) in n_chunks:
                    wsb = fcwp.tile([128, 2 * KH, 512], f32r, tag="fcw", name="fcw_sb")
                    nc.sync.dma_start(wsb[:, :, 0:nsz], fcWT[:, :, n0:n0 + nsz])
                    for m in range(mt):
                        ps = fps.tile([128, 512], f32, tag="fps", name="fps_t")
                        for k in range(KH):
                            nc.tensor.matmul(
                                ps[:, 0:nsz], htf[:, k, m * 128:(m + 1) * 128],
                                wsb[:, k, 0:nsz], start=(k == 0), stop=False)
                        for k in range(KH):
                            nc.tensor.matmul(
                                ps[:, 0:nsz], htb[:, k, m * 128:(m + 1) * 128],
                                wsb[:, KH + k, 0:nsz], start=False, stop=(k == KH - 1))
                        osb = fcop.tile([128, 512], f32, tag="fco", name="fco_t")
                        nc.vector.tensor_add(out=osb[:, 0:nsz], in0=ps[:, 0:nsz],
                                             in1=fcb_sb[:, n0:n0 + nsz])
                        nc.sync.dma_start(
                            logits[m * 128:(m + 1) * 128, n0:n0 + nsz], osb[:, 0:nsz])
    nc.compile()
    return nc


def _get_nc(nc_T):
    if nc_T not in _CACHE:
        _CACHE[nc_T] = _build(nc_T)
    return _CACHE[nc_T]


def _kxm(x):
    """[K, M] -> [128, K/128, M] (partition-inner K)."""
    K, M = x.shape
    return np.ascontiguousarray(x.reshape(K // 128, 128, M).transpose(1, 0, 2))


def _prep_inputs(x, embed, weights, fc_W, fc_b, nc_T):
    x = np.asarray(x).astype(np.int64)
    emb = np.asarray(embed)[x]                      # [B, T, E]
    emb_tb = np.ascontiguousarray(emb.transpose(1, 0, 2))[:nc_T]   # [T, B, E]
    embT_f = _kxm(np.ascontiguousarray(emb_tb.reshape(nc_T * B, E).T))
    emb_rev = np.ascontiguousarray(emb_tb[::-1])
    embT_b = _kxm(np.ascontiguousarray(emb_rev.reshape(nc_T * B, E).T))

    base = {
        "embT_f": embT_f, "embT_b": embT_b,
        "i8r": np.eye(8, dtype=np.float32), "i8f": np.eye(8, dtype=np.float32),
        "zcol": np.zeros((128, KH, 8), dtype=np.float32),
    }
    for c, (Wih, bih_v, Whh) in weights.items():
        base[f"wihT_{c}"] = _kxm(np.ascontiguousarray(Wih.T))
        base[f"whhT_{c}"] = _kxm(np.ascontiguousarray(Whh.T))
        base[f"bih_{c}"] = np.ascontiguousarray(
            np.broadcast_to(bih_v[None, :], (128, G))).astype(np.float32)

    fcWT = np.ascontiguousarray(fc_W.T)     # [2H, V]
    in_maps = []
    for core in range(NCORES):
        m = dict(base)
        sh = slice(core * VSH, (core + 1) * VSH)
        m["fcWT"] = _kxm(np.ascontiguousarray(fcWT[:, sh]))
        m["fcb"] = np.ascontiguousarray(
            np.broadcast_to(fc_b[None, sh], (128, VSH))).astype(np.float32)
        in_maps.append(m)
    return in_maps


def kernel(x, embed,
           fwd0_Wih, fwd0_bih, fwd0_Whh, fwd1_Wih, fwd1_bih, fwd1_Whh,
           bwd0_Wih, bwd0_bih, bwd0_Whh, bwd1_Wih, bwd1_bih, bwd1_Whh,
           fc_W, fc_b):
    from concourse.bass_utils import run_bass_kernel_spmd

    nc_T = int(os.environ.get("KERNEL_T", T))
    nc = _get_nc(nc_T)
    weights = {
        "f0": (np.asarray(fwd0_Wih), np.asarray(fwd0_bih), np.asarray(fwd0_Whh)),
        "f1": (np.asarray(fwd1_Wih), np.asarray(fwd1_bih), np.asarray(fwd1_Whh)),
        "b0": (np.asarray(bwd0_Wih), np.asarray(bwd0_bih), np.asarray(bwd0_Whh)),
        "b1": (np.asarray(bwd1_Wih), np.asarray(bwd1_bih), np.asarray(bwd1_Whh)),
    }
    in_maps = _prep_inputs(x, embed, weights, np.asarray(fc_W), np.asarray(fc_b), nc_T)
    res = run_bass_kernel_spmd(nc, in_maps, list(range(NCORES)))

    logits = np.concatenate([r["logits"] for r in res.results], axis=1)  # [(t b), V]
    logits = logits.reshape(nc_T, B, V)
    logits = np.ascontiguousarray(logits.transpose(1, 0, 2))             # [B, T, V]
    hT = res.results[0]["hT_o"]
    cT = res.results[0]["cT_o"]
    return logits.astype(np.float32), hT.astype(np.float32), cT.astype(np.float32)
